# revision 1
# baseline (speedup 1.0000x reference)
"""AttentionBlock (GroupNorm + 1x1-conv QKV + full NxN attention + proj +
residual) on 8 Trainium2 NeuronCores, data-parallel over the batch dim.

Per core: 2 samples of x[16, 512, 32, 32]. Matmul operands are fp8e4m3
with DoubleRow perf mode. PSUM accumulation and the residual stay fp32.

Algebraic folds (host, exact up to fp8 rounding):
  - GroupNorm affine folded into qkv_w / qkv_b.
  - Q and K projections merged: S = xn^T (Wq^T Wk) xn, so only one
    projection G = (16 Wq^T Wk)^T xn is computed on-chip (the x16 keeps
    the small A = Wq^T Wk entries out of fp8 subnormals; compensated in
    the exp scale). The Q-side bias adds a per-i constant to the logits
    (cancels in softmax exactly); the K-side bias term is O(1e-2) of the
    logit std and is dropped (validated: l2rel 8.6e-4 vs 9.0e-4 with it).
  - V bias and proj bias folded into one per-channel bias
    pb = proj_w @ bv + proj_b (softmax rows sum to 1), applied in the
    fused proj evacuation out = (psum + pb) + x (scalar_tensor_tensor).
  - softmax denominator folded through the projection via the O
    evacuation: O = (V @ E^T) * (1/Z), 1/Z = exp(-ln Z) on ACT.

Schedule notes (evacuation bandwidth is the co-bottleneck: a [128,512]
PSUM->SBUF pass costs ~830ns on either ACT or DVE):
  - All matmul outputs use two-bank PSUM tiles [128, 2, 512] and are
    evacuated in ONE [128,1024] instruction (exp / copy / mult), halving
    the per-instruction overhead and instruction count.
  - Warmup matmuls rotate through the 3-buffer mm pool (not a 1-buffer
    WAR chain) so they pipeline during the DMA/GroupNorm head and the
    HAM clock-gate is warm before real work.
  - S phases are exp-throughput-limited on ACT, so independent PE work
    is interleaved into them (O/proj of the previous i-block).
  - Sample 1's GN stats run on DVE during sample 0's S phase; its chain
    matmuls are emitted after the S-ib0 block (so they never head-block
    the PE queue); its GN apply is split DVE/ACT into known engine gaps.
"""

import math
import sys

import numpy as np

try:
    import concourse.bass as bass
except ImportError:  # pragma: no cover - grading container path setup
    sys.path.insert(0, "/opt/trn_rl_repo")
    import concourse.bass as bass

import bass_rust
import ml_dtypes
import concourse.tile as tile
from concourse import mybir
from concourse.bass_utils import run_bass_kernel_spmd

F32 = mybir.dt.float32
BF16 = mybir.dt.bfloat16
FP8 = mybir.dt.float8e4
DR = mybir.MatmulPerfMode.DoubleRow
AF = mybir.ActivationFunctionType
OP = mybir.AluOpType

NCORES = 8
B = 16
S = B // NCORES  # samples per core
C = 512
N = 1024  # H*W
G = 8  # groups
EPS = 1e-5
CT = C // 128  # channel p-tiles (4)
NT = N // 128  # spatial p-tiles (8)
IBS = 512  # i-block size
IB = N // IBS  # i blocks (2)
A_SCALE = 16.0  # host scaling of A = Wq^T Wk for fp8 range
EXP_SCALE = 1.0 / (A_SCALE * math.sqrt(C))
NWARM = 30
NFILL = 8  # low-priority HAM fillers across the GN-apply stall window  # HAM warmup matmuls — bridge the whole DMA/GroupNorm head

# Settable by test harness for profiling; not used by the grader.
TRACE = False
LAST_RESULT = None


MAX_WAITS = 1


def _split_excess_waits(nc, max_waits=MAX_WAITS):
    """Workaround for a walrus codegen limit: an instruction may carry at
    most `max_waits` semaphore waits ("Too many sync wait commands").
    Move the excess onto a chain of NOPs on the same engine right before
    the instruction — sequentially blocking waits on one engine queue are
    semantically identical to one multi-wait instruction."""
    counter = 0
    for f in nc.m.functions:
        for blk in f.blocks:
            il = blk.instructions
            if not any(
                i.sync_info is not None and len(i.sync_info.on_wait) > max_waits
                for i in il
            ):
                continue
            old = list(il)
            il.clear()
            for ins in old:
                si = ins.sync_info
                waits = list(si.on_wait) if si is not None else []
                if len(waits) > max_waits:
                    excess, keep = waits[:-max_waits], waits[-max_waits:]
                    for i0 in range(0, len(excess), max_waits):
                        counter += 1
                        nop = mybir.InstNoOp(
                            name=f"waitsplit-{counter}",
                            engine=ins.engine,
                            ins=[],
                            outs=[],
                            sync_info=bass_rust.SyncInfo(
                                on_wait=excess[i0 : i0 + max_waits], on_update=[]
                            ),
                        )
                        nc.register_instruction(nop, overwrite=True)
                        blk.add_instruction(nop)
                    ins.sync_info = bass_rust.SyncInfo(
                        on_wait=keep, on_update=list(si.on_update)
                    )
                blk.add_instruction(ins)
    return counter


def _build():
    from contextlib import ExitStack

    nc = bass.Bass()
    xd = nc.declare_dram_parameter("x", [S, C, N], F32, isOutput=False)
    wald = nc.declare_dram_parameter("wall", [128, 2 * CT, C], FP8, isOutput=False)
    pbd = nc.declare_dram_parameter("pball", [128, CT], F32, isOutput=False)
    cad = nc.declare_dram_parameter("ca", [128, 2], BF16, isOutput=False)
    cbd = nc.declare_dram_parameter("cb", [2, 128], BF16, isOutput=False)
    outd = nc.declare_dram_parameter("out", [S, C, N], F32, isOutput=True)

    x_ap = xd[:].rearrange("s (t p) n -> s t p n", p=128)
    out_ap = outd[:].rearrange("s (t p) n -> s p t n", p=128)

    with tile.TileContext(nc) as tc, ExitStack() as ctx:
        singles = ctx.enter_context(tc.tile_pool(name="singles", bufs=1))
        xp = ctx.enter_context(tc.tile_pool(name="xp", bufs=S))
        xnp = ctx.enter_context(tc.tile_pool(name="xnp", bufs=2 * S))
        gp = ctx.enter_context(tc.tile_pool(name="gp", bufs=2))
        vp = ctx.enter_context(tc.tile_pool(name="vp", bufs=2))
        ep = ctx.enter_context(tc.tile_pool(name="ep", bufs=12))
        tmpp = ctx.enter_context(tc.tile_pool(name="tmpp", bufs=2))
        rzp = ctx.enter_context(tc.tile_pool(name="rzp", bufs=2))
        zbp = ctx.enter_context(tc.tile_pool(name="zbp", bufs=2))
        obp = ctx.enter_context(tc.tile_pool(name="obp", bufs=3))
        smp = ctx.enter_context(tc.tile_pool(name="smp", bufs=4))
        ps = ctx.enter_context(tc.tile_pool(name="ps", bufs=1, space="PSUM"))

        # ---- input DMAs. Only sync and scalar are hardware DGE rings
        # (the gpsimd ring is software DGE — slow; it only gets the
        # output writes). Transfers stripe across all 16 DMA engines, so
        # ring order IS priority order: sample-0 x tiles first split
        # across both rings, then weights, then sample-1 x tiles. ----
        ca = singles.tile([128, 2], BF16)
        nc.sync.dma_start(out=ca[:], in_=cad[:])
        cb = singles.tile([2, 128], BF16)
        nc.sync.dma_start(out=cb[:], in_=cbd[:])
        xs = []
        for s in range(S):
            x_sb = xp.tile([128, CT, N], F32, tag="x", name=f"x{s}")
            xs.append(x_sb)
        wall = singles.tile([128, 2 * CT, C], FP8)
        nc.scalar.dma_start(out=wall[:], in_=wald[:])
        pball = singles.tile([128, CT], F32)
        nc.scalar.dma_start(out=pball[:], in_=pbd[:])
        for t in range(CT):
            nc.sync.dma_start(out=xs[0][:, t, 0:512], in_=x_ap[0, t][:, 0:512])
            nc.sync.dma_start(out=xs[0][:, t, 512:1024], in_=x_ap[0, t][:, 512:1024])
        for t in range(CT):
            nc.sync.dma_start(out=xs[1][:, t, :], in_=x_ap[1, t])
        ones8 = singles.tile([128, 2, 16], FP8)
        nc.vector.memset(ones8[:], 1.0)
        ones_mov = singles.tile([128, 2, 512], FP8)
        nc.vector.memset(ones_mov[:], 1.0)
        eps_sb = singles.tile([2, 1], F32)
        nc.vector.memset(eps_sb[:], EPS)
        nl16_sb = singles.tile([1, 1], F32)
        nc.vector.memset(nl16_sb[:], -math.log(A_SCALE))
        ones_row = singles.tile([1, 128], BF16)
        nc.vector.memset(ones_row[:], 1.0)
        # weight planes for DoubleRow: [p, wi, g, q, o]; channel = 256g+128q+p
        w8 = wall.rearrange("p (w g q) f -> p w g q f", g=2, q=2)
        ghot, hhot = ca[:, 0:2], cb[:, :]

        def mm2(name):
            return ps.tile([128, 2, IBS], F32, tag="mm", bufs=3, name=name)

        # ---- PE warmup through the rotating mm pool: dense back-to-back
        # matmuls so the HAM clock-gate actually reaches 2.4 GHz during the
        # DMA/GroupNorm head (sparse warmups never trip its busy window) ----
        for i in range(NWARM):
            zw = mm2(f"warm{i}")
            nc.tensor.matmul(
                zw[0:1, 0, :], lhsT=ones8[:, :, 0:1], rhs=ones_mov[:],
                start=True, stop=True, perf_mode=DR,
            )

        xns = [None] * S
        sas = [None] * S

        def emit_stats(s, act_t0):
            x_sb = xs[s]
            sa_h = [
                smp.tile([128, 4], BF16, tag="sa", name=f"sa{s}{h}")
                for h in range(2)
            ]
            sas[s] = sa_h
            # sa columns per tile: (-mean, E[x^2]) — negated mean makes the
            # apply a*x + b on either DVE or ACT.

            def dve_stats(t):
                sa = sa_h[t // 2]
                st6 = smp.tile([128, 2, 6], F32, tag="st6")
                nc.vector.bn_stats(out=st6[:, 0, :], in_=x_sb[:, t, 0:512])
                nc.vector.bn_stats(out=st6[:, 1, :], in_=x_sb[:, t, 512:1024])
                mv = smp.tile([128, 2], F32, tag="mv")
                nc.vector.bn_aggr(out=mv[:], in_=st6[:])
                nc.vector.tensor_scalar(
                    out=sa[:, 2 * (t % 2) : 2 * (t % 2) + 1],
                    in0=mv[:, 0:1],
                    scalar1=-1.0,
                    scalar2=None,
                    op0=OP.mult,
                )
                msq = smp.tile([128, 1], F32, tag="msq")
                nc.vector.tensor_mul(msq[:], mv[:, 0:1], mv[:, 0:1])
                nc.vector.tensor_tensor(
                    out=sa[:, 2 * (t % 2) + 1 : 2 * (t % 2) + 2],
                    in0=mv[:, 1:2],
                    in1=msq[:],
                    op=OP.add,
                )

            def act_stats(t):
                sa = sa_h[t // 2]
                scr = smp.tile([128, N], BF16, tag="scr")
                sum3 = smp.tile([128, 1], F32, tag="sum3")
                nc.scalar.activation(
                    out=scr[:], in_=x_sb[:, t, :], func=AF.Identity,
                    accum_out=sum3[:],
                )
                scr2 = smp.tile([128, N], BF16, tag="scr")
                sq3 = smp.tile([128, 1], F32, tag="sq3")
                nc.scalar.activation(
                    out=scr2[:], in_=x_sb[:, t, :], func=AF.Square,
                    accum_out=sq3[:],
                )
                c0 = 2 * (t % 2)
                nc.scalar.activation(
                    out=sa[:, c0 : c0 + 1], in_=sum3[:], func=AF.Identity,
                    scale=-1.0 / N,
                )
                nc.scalar.activation(
                    out=sa[:, c0 + 1 : c0 + 2], in_=sq3[:], func=AF.Identity,
                    scale=1.0 / N,
                )

            if act_t0:
                act_stats(0)
                dve_stats(1)
                dve_stats(2)
                dve_stats(3)
            else:
                dve_stats(0)
                dve_stats(1)
                dve_stats(2)
                dve_stats(3)

        def emit_chain(s, h0, apply_eng):
            """Group reduce -> rstd -> broadcast -> apply for tile pair h0.
            apply_eng: list of 'dve'/'act'/None per tile in the pair (None =
            deferred; caller emits the apply later via emit_apply)."""
            x_sb = xs[s]
            sa_h = sas[s]
            if xns[s] is None:
                xns[s] = [
                    xnp.tile(
                        [128, 2, N], FP8, tag="xn", bufs=2 * S, name=f"xn{s}{g}"
                    )
                    for g in range(2)
                ]
            xn_g = xns[s]
            gs_ps = ps.tile([2, 4], F32, tag="gn", bufs=1, name=f"gs{s}{h0}")
            nc.tensor.matmul(
                gs_ps[:], lhsT=ghot, rhs=sa_h[h0][:], start=True, stop=True
            )
            gs3 = gs_ps.rearrange("h (t s) -> h t s", s=2)
            # gs3[:,:,0] = -mean_g ; gs3[:,:,1] = E[x^2]_g
            sq = smp.tile([2, 2], F32, tag="sq")
            nc.scalar.activation(out=sq[:], in_=gs3[:, :, 0], func=AF.Square)
            var = smp.tile([2, 2], F32, tag="var")
            nc.vector.tensor_tensor(
                out=var[:], in0=gs3[:, :, 1], in1=sq[:], op=OP.subtract
            )
            lnv = smp.tile([2, 2], F32, tag="lnv")
            nc.scalar.activation(
                out=lnv[:], in_=var[:], func=AF.Ln, bias=eps_sb[:], scale=1.0
            )
            # vals: (rstd, -mean*rstd); apply is x*rstd + (-mean*rstd)
            vals = smp.tile([2, 4], BF16, tag="vals")
            vals3 = vals.rearrange("h (t s) -> h t s", s=2)
            nc.scalar.activation(
                out=vals3[:, :, 0], in_=lnv[:], func=AF.Exp, scale=-0.5
            )
            nc.vector.tensor_tensor(
                out=vals3[:, :, 1], in0=gs3[:, :, 0], in1=vals3[:, :, 0],
                op=OP.mult,
            )
            bc = ps.tile([128, 4], F32, tag="gn", bufs=1, name=f"bc{s}{h0}")
            nc.tensor.matmul(bc[:], lhsT=hhot, rhs=vals[:], start=True, stop=True)
            bcs = smp.tile([128, 4], F32, tag="bcs", name=f"bcs{s}{h0}")
            nc.vector.tensor_copy(out=bcs[:], in_=bc[:])
            for tt in range(2):
                if apply_eng[tt] is not None:
                    emit_apply(s, h0, tt, bcs, apply_eng[tt])
            return bcs

        def emit_apply(s, h0, tt, bcs, eng):
            x_sb = xs[s]
            xn_g = xns[s]
            t = 2 * h0 + tt
            if eng == "act":
                nc.scalar.activation(
                    out=xn_g[h0][:, tt, :],
                    in_=x_sb[:, t, :],
                    func=AF.Identity,
                    bias=bcs[:, 2 * tt + 1 : 2 * tt + 2],
                    scale=bcs[:, 2 * tt : 2 * tt + 1],
                )
            else:
                nc.vector.tensor_scalar(
                    out=xn_g[h0][:, tt, :],
                    in0=x_sb[:, t, :],
                    scalar1=bcs[:, 2 * tt : 2 * tt + 1],
                    scalar2=bcs[:, 2 * tt + 1 : 2 * tt + 2],
                    op0=OP.mult,
                    op1=OP.add,
                )

        gvs = [None] * S

        def emit_gv(s):
            """G and V projections, G/V rounds interleaved so the PE is the
            pacing engine while G evacs go to ACT and V evacs to DVE."""
            xn_g = xns[s]
            g_sb = gp.tile([128, 2, 2, N], FP8, tag="g")
            v_sb = vp.tile([128, NT // 2, 2, C], FP8, tag="v")
            gvs[s] = (g_sb, v_sb)
            gv = g_sb.rearrange("p g q n -> p (g q) n")
            vv = v_sb.rearrange("p g q n -> p (g q) n")
            for r in range(CT):
                # G tile: both i-halves of output row-block r
                psg = mm2(f"gps{s}{r}")
                for g in range(2):
                    for ib in range(IB):
                        nc.tensor.matmul(
                            psg[:, ib, :],
                            lhsT=w8[:, 0, g, :, r * 128 : (r + 1) * 128],
                            rhs=xn_g[g][:, :, ib * IBS : (ib + 1) * IBS],
                            start=(g == 0),
                            stop=(g == 1),
                            perf_mode=DR,
                        )
                nc.scalar.activation(out=gv[:, r, :], in_=psg[:], func=AF.Identity)
                # V pair: spatial tiles 2r, 2r+1
                psv = mm2(f"vps{s}{r}")
                for h in range(2):
                    nt = 2 * r + h
                    for g in range(2):
                        nc.tensor.matmul(
                            psv[:, h, :],
                            lhsT=xn_g[g][:, :, nt * 128 : (nt + 1) * 128],
                            rhs=w8[:, 1, g, :, :],
                            start=(g == 0),
                            stop=(g == 1),
                            perf_mode=DR,
                        )
                nc.vector.tensor_copy(out=vv[:, 2 * r : 2 * r + 2, :], in_=psv[:])

        def emit_s_pair(s, ib, jp, es, zps=None):
            """S psum pair (jt = 2*jp, 2*jp+1) for i-block ib + one exp.
            When `zps` is given, the Z row-sum matmul for this pair rides
            immediately behind the exp (so Z completes with the last exp
            and the 1/Z chain never gates the output stage)."""
            xn_g = xns[s]
            g_sb, _ = gvs[s]
            isl = slice(ib * IBS, (ib + 1) * IBS)
            psm = mm2(f"sps{s}{ib}{jp}")
            for h in range(2):
                jt = 2 * jp + h
                for g in range(2):
                    nc.tensor.matmul(
                        psm[:, h, :],
                        lhsT=xn_g[g][:, :, jt * 128 : (jt + 1) * 128],
                        rhs=g_sb[:, g, :, isl],
                        start=(g == 0),
                        stop=(g == 1),
                        perf_mode=DR,
                    )
            e = ep.tile([128, 2, IBS], FP8, tag="e", name=f"e{s}{ib}{jp}")
            es.append(e)
            nc.scalar.activation(out=e[:], in_=psm[:], func=AF.Exp, scale=EXP_SCALE)
            if zps is not None:
                nc.tensor.matmul(
                    zps[:],
                    lhsT=ones8[:, :, 0:1],
                    rhs=e[:],
                    start=(jp == 0),
                    stop=(jp == NT // 2 - 1),
                    perf_mode=DR,
                )

        def emit_rz(s, ib, zps):
            with tc.high_priority():
                lnz = rzp.tile([1, IBS], F32, tag="lnz", name=f"lnz{s}{ib}")
                nc.scalar.activation(out=lnz[:], in_=zps[:], func=AF.Ln)
                rz = rzp.tile([1, IBS], BF16, tag="rz", name=f"rz{s}{ib}")
                nc.scalar.activation(
                    out=rz[:], in_=lnz[:], func=AF.Exp, scale=-1.0,
                    bias=nl16_sb[:],
                )
                zb_ps = ps.tile([128, IBS], F32, tag="gn", bufs=1, name=f"zbp{s}{ib}")
                nc.tensor.matmul(
                    zb_ps[:], lhsT=ones_row, rhs=rz[:], start=True, stop=True
                )
                zb = zbp.tile([128, IBS], F32, tag="zb", name=f"zb{s}{ib}")
                nc.vector.tensor_copy(out=zb[:], in_=zb_ps[:])
            return zb

        def emit_out_pair(s, ib, cp, es, zb, ob4, last):
            # U @ E^T for channel pair (2cp, 2cp+1), then the full output:
            # out = (psum * zb + pb) + x in two evacuation passes
            x_sb = xs[s]
            _, u_sb = gvs[s]
            isl = slice(ib * IBS, (ib + 1) * IBS)
            psm = mm2(f"ops{s}{ib}{cp}")
            for h in range(2):
                ct = 2 * cp + h
                for jg in range(NT // 2):
                    nc.tensor.matmul(
                        psm[:, h, :],
                        lhsT=u_sb[:, jg, :, ct * 128 : (ct + 1) * 128],
                        rhs=es[jg][:],
                        start=(jg == 0),
                        stop=(jg == NT // 2 - 1),
                        perf_mode=DR,
                    )
            tmp = tmpp.tile([128, 2, IBS], F32, tag="tmp", name=f"tmp{s}{ib}{cp}")
            nc.vector.tensor_tensor(
                out=tmp[:],
                in0=psm[:],
                in1=zb[:, None, :].broadcast_to([128, 2, IBS]),
                op=OP.mult,
            )
            for h in range(2):
                ot = 2 * cp + h
                nc.vector.scalar_tensor_tensor(
                    out=ob4[:, ot, :],
                    in0=tmp[:, h, :],
                    scalar=pball[:, ot : ot + 1],
                    in1=x_sb[:, ot, isl],
                    op0=OP.add,
                    op1=OP.add,
                )
                if last:
                    eng = nc.sync if ot % 2 == 0 else nc.scalar
                    eng.dma_start(
                        out=out_ap[s][:, ot : ot + 1, isl],
                        in_=ob4[:, ot : ot + 1, :],
                    )

        def emit_out_tail(s, es, zb, ob4):
            # final i-block: per-ct single-bank chains so each 256KB output
            # chunk evacuates and ships as soon as its matmuls finish
            x_sb = xs[s]
            _, u_sb = gvs[s]
            isl = slice(IBS, N)
            for ct in range(CT):
                psm = mm2(f"tps{s}{ct}")
                for jg in range(NT // 2):
                    nc.tensor.matmul(
                        psm[:, 0, :],
                        lhsT=u_sb[:, jg, :, ct * 128 : (ct + 1) * 128],
                        rhs=es[jg][:],
                        start=(jg == 0),
                        stop=(jg == NT // 2 - 1),
                        perf_mode=DR,
                    )
                tmp = tmpp.tile([128, 2, IBS], F32, tag="tmp", name=f"ttm{s}{ct}")
                nc.vector.tensor_tensor(
                    out=tmp[:, 0, :], in0=psm[:, 0, :], in1=zb[:], op=OP.mult
                )
                nc.vector.scalar_tensor_tensor(
                    out=ob4[:, ct, :],
                    in0=tmp[:, 0, :],
                    scalar=pball[:, ct : ct + 1],
                    in1=x_sb[:, ct, isl],
                    op0=OP.add,
                    op1=OP.add,
                )
                eng = nc.sync if ct % 2 == 0 else nc.scalar
                eng.dma_start(
                    out=out_ap[s][:, ct : ct + 1, isl],
                    in_=ob4[:, ct : ct + 1, :],
                )

        def emit_attn(s, mid=None):
            """Attention for sample s. `mid` is an optional callback emitted
            after the S-ib0 block (used to slot sample-1 GN chain work into
            a known PE/ACT gap)."""
            es_ib = [[], []]
            zps0 = ps.tile([1, IBS], F32, tag="z", bufs=1, name=f"z{s}0")
            # S for i-block 0 (Z row-sums ride each exp)
            for jp in range(NT // 2):
                emit_s_pair(s, 0, jp, es_ib[0], zps=zps0)
            if mid is not None:
                mid()
            zb0 = emit_rz(s, 0, zps0)
            ob40 = obp.tile([128, CT, IBS], F32, tag="ob", bufs=3, name=f"ob{s}0")
            zps1 = ps.tile([1, IBS], F32, tag="z", bufs=1, name=f"z{s}1")
            # S for i-block 1 interleaved with the output stage of i-block 0
            # (S is exp-limited on ACT; U@E^T keeps the PE busy meanwhile)
            emit_s_pair(s, 1, 0, es_ib[1], zps=zps1)
            emit_out_pair(s, 0, 0, es_ib[0], zb0, ob40, last=False)
            emit_s_pair(s, 1, 1, es_ib[1], zps=zps1)
            emit_s_pair(s, 1, 2, es_ib[1], zps=zps1)
            emit_out_pair(s, 0, 1, es_ib[0], zb0, ob40, last=False)
            emit_s_pair(s, 1, 3, es_ib[1], zps=zps1)
            nc.sync.dma_start(out=out_ap[s][:, :, 0:IBS], in_=ob40[:])
            # i-block 1 tail
            zb1 = emit_rz(s, 1, zps1)
            ob41 = obp.tile([128, CT, IBS], F32, tag="ob", bufs=3, name=f"ob{s}1")
            yield  # let the caller emit fill work here (gv of next sample)
            last = s == S - 1
            if not last:
                emit_out_pair(s, 1, 0, es_ib[1], zb1, ob41, last=False)
                emit_out_pair(s, 1, 1, es_ib[1], zb1, ob41, last=False)
                nc.scalar.dma_start(out=out_ap[s][:, :, IBS:N], in_=ob41[:])
            else:
                emit_out_tail(s, es_ib[1], zb1, ob41)

        # ---------------- emission schedule ----------------
        emit_stats(0, act_t0=True)
        # the s0 chain+apply gates every matmul — pin it ahead of anything
        # the scheduler might greedily slot into the same engine windows
        with tc.high_priority():
            emit_chain(0, 0, apply_eng=("act", "dve"))
            emit_chain(0, 1, apply_eng=("dve", "act"))
        emit_gv(0)

        bcs1 = [None, None]

        def mid0():
            # sample-1 GN stats + chain: the stats run in s0's S-phase DVE
            # idle window (deferring them keeps them out of the congested
            # s0 G/V window); PE one-hots land in the post-S-ib0 gap
            emit_stats(1, act_t0=False)
            bcs1[0] = emit_chain(1, 0, apply_eng=(None, None))
            bcs1[1] = emit_chain(1, 1, apply_eng=(None, None))
            # DVE has slack during s0's S/out phases
            emit_apply(1, 0, 0, bcs1[0], "dve")
            emit_apply(1, 0, 1, bcs1[0], "dve")

        a0 = emit_attn(0, mid=mid0)
        next(a0)
        # ACT is free of exps here (s0 exps done); finish s1's apply
        emit_apply(1, 1, 0, bcs1[1], "act")
        emit_apply(1, 1, 1, bcs1[1], "act")
        emit_gv(1)
        for _ in a0:
            pass
        a1 = emit_attn(1)
        next(a1)
        for _ in a1:
            pass

    _split_excess_waits(nc)
    return nc


_NC = None


def kernel(x, norm_w, norm_b, qkv_w, qkv_b, proj_w, proj_b):
    global _NC, LAST_RESULT
    x = np.ascontiguousarray(np.asarray(x, dtype=np.float32))
    norm_w = np.asarray(norm_w, dtype=np.float32)
    norm_b = np.asarray(norm_b, dtype=np.float32)
    qkv_w = np.asarray(qkv_w, dtype=np.float32)
    qkv_b = np.asarray(qkv_b, dtype=np.float32)
    proj_w = np.asarray(proj_w, dtype=np.float32)
    proj_b = np.asarray(proj_b, dtype=np.float32)

    # fold GroupNorm affine into qkv
    wq_full = qkv_w * norm_w[None, :]
    bq_full = qkv_b + qkv_w @ norm_b
    wq_, wk_, wv_ = wq_full[0:C], wq_full[C : 2 * C], wq_full[2 * C : 3 * C]
    bv_ = bq_full[2 * C : 3 * C]
    pb_ = proj_w @ bv_ + proj_b
    # merged Q/K projection weight (scaled into fp8 range)
    a_w = (A_SCALE * (wq_.T @ wk_)).T  # G = a_w^T @ xn, i.e. "w" = A'.T
    # proj folded into V: out = x + pb + (proj_w @ Wv) xn attn^T
    u_w = A_SCALE * (proj_w @ wv_)

    def wtile(w):  # [o, c] -> DoubleRow lhsT planes [128, 2(g), 2(q), o]
        return w.T.reshape(2, 2, 128, C).transpose(2, 0, 1, 3)

    def btile(b):  # [C] -> [128, ct]
        return b.reshape(CT, 128).T

    wall = np.ascontiguousarray(
        np.stack(
            [wtile(a_w), wtile(u_w)], axis=1
        ).reshape(128, 8, C).astype(ml_dtypes.float8_e4m3)
    )
    pball = np.ascontiguousarray(btile(pb_).astype(np.float32))
    cl = np.arange(128)
    ghot = np.zeros((128, 2), np.float32)
    ghot[cl, cl // 64] = 1.0 / 64.0
    hhot = np.zeros((2, 128), np.float32)
    hhot[cl // 64, cl] = 1.0

    common = {
        "wall": wall,
        "pball": pball,
        "ca": ghot.astype(ml_dtypes.bfloat16),
        "cb": hhot.astype(ml_dtypes.bfloat16),
    }
    xr = x.reshape(NCORES, S, C, N)
    in_maps = [dict(common, x=np.ascontiguousarray(xr[i])) for i in range(NCORES)]

    if _NC is None:
        _NC = _build()
    res = run_bass_kernel_spmd(
        _NC, in_maps, core_ids=list(range(NCORES)), trace=TRACE
    )
    LAST_RESULT = res
    out = np.stack([res.results[i]["out"] for i in range(NCORES)])
    return np.ascontiguousarray(out.reshape(B, C, 32, 32).astype(np.float32))

